# revision 1
# baseline (speedup 1.0000x reference)
"""KGATConv GNN message-passing kernel for 8 Trainium2 NeuronCores.

Strategy (dst-node ownership, no collectives):
  - Core k owns nodes [k*12500, (k+1)*12500).
  - Host sorts edges by dst and buckets per (core, 128-node window), padding
    each window's edge run to whole 128-edge chunks (chunk counts shared
    across cores so all 8 run one SPMD program).
  - Device, per chunk: indirect-DMA gather of 128 nfeat[src] rows (one offset
    per partition -- the only indirect mode this toolchain executes
    correctly); DVE builds A[p,j] = w_p * (dst_p == j); PE matmul-accumulates
    h_nb = A^T @ msg in PSUM.  Finalize per window: X = nfeat_own * h_nb,
    X^T via PE transpose, out = X @ W^T on PE, LeakyReLU on ACT, DMA out.
"""

import sys

sys.path.insert(0, "/opt/trn_rl_repo")

from contextlib import ExitStack

import numpy as np

import concourse.bass as bass
import concourse.mybir as mybir
import concourse.tile as tile
from concourse.bass_utils import run_bass_kernel_spmd

N_CORES = 8
D = 128
WIN = 128

_nc_cache = {}


def _split_excess_waits(nc, maxw=1):
    # This walrus build rejects instructions carrying more than one sync
    # wait; move extras onto preceding single-wait NoOps on the same engine.
    for f in nc.m.functions:
        for bb in f.blocks:
            out = []
            for inst in bb.instructions:
                si = inst.sync_info
                waits = list(si.on_wait) if si and si.on_wait else []
                if len(waits) > maxw:
                    extra, keep = waits[:-maxw], waits[-maxw:]
                    for i in range(0, len(extra), maxw):
                        nop = mybir.InstNoOp(
                            name=nc.get_next_instruction_name(), ins=[], outs=[]
                        )
                        nop.engine = inst.engine
                        nop.sync_info = type(si)(
                            on_wait=extra[i : i + maxw], on_update=[]
                        )
                        nc.register_instruction(nop, overwrite=True)
                        out.append(nop)
                    si.on_wait = keep
                out.append(inst)
            bb.instructions[:] = out


def _build_nc(n_rows, nw, ct, c_list):
    f32 = mybir.dt.float32
    nc = bass.Bass()
    nfeat_d = nc.declare_dram_parameter("nfeat", [n_rows, D], f32, isOutput=False)
    nfown_d = nc.declare_dram_parameter("nfown", [nw * WIN, D], f32, isOutput=False)
    src_d = nc.declare_dram_parameter("src", [128, ct], mybir.dt.int32, isOutput=False)
    dst_d = nc.declare_dram_parameter("dstf", [128, ct], f32, isOutput=False)
    w_d = nc.declare_dram_parameter("wf", [128, ct], f32, isOutput=False)
    wt_d = nc.declare_dram_parameter("wt", [D, D], f32, isOutput=False)
    iota_d = nc.declare_dram_parameter("iota", [128, WIN], f32, isOutput=False)
    ident_d = nc.declare_dram_parameter("ident", [128, 128], f32, isOutput=False)
    out_d = nc.declare_dram_parameter("out", [nw * WIN, D], f32, isOutput=True)

    with tile.TileContext(nc) as tc, ExitStack() as ctx:
        const = ctx.enter_context(tc.tile_pool(name="const", bufs=1))
        gp = ctx.enter_context(tc.tile_pool(name="gp", bufs=10))
        ap = ctx.enter_context(tc.tile_pool(name="ap", bufs=4))
        wk = ctx.enter_context(tc.tile_pool(name="wk", bufs=3))
        ps = ctx.enter_context(tc.tile_pool(name="ps", bufs=2, space="PSUM"))

        src_sb = const.tile([128, ct], mybir.dt.int32)
        nc.sync.dma_start(out=src_sb[:], in_=src_d[:])
        dst_sb = const.tile([128, ct], f32)
        nc.sync.dma_start(out=dst_sb[:], in_=dst_d[:])
        w_sb = const.tile([128, ct], f32)
        nc.sync.dma_start(out=w_sb[:], in_=w_d[:])
        wt_sb = const.tile([D, D], f32)
        nc.sync.dma_start(out=wt_sb[:], in_=wt_d[:])
        iota_sb = const.tile([128, WIN], f32)
        nc.sync.dma_start(out=iota_sb[:], in_=iota_d[:])
        ident_sb = const.tile([128, 128], f32)
        nc.sync.dma_start(out=ident_sb[:], in_=ident_d[:])

        start = 0
        for t in range(nw):
            c = c_list[t]
            acc = ps.tile([WIN, D], f32, tag="acc")
            for j in range(c):
                col = start + j
                # one offset per partition; dest [128,128] = one nfeat row
                # per partition (the only indirect mode this walrus build
                # executes correctly).
                g = gp.tile([128, D], f32, tag="g")
                nc.gpsimd.indirect_dma_start(
                    out=g[:],
                    out_offset=None,
                    in_=nfeat_d[:],
                    in_offset=bass.IndirectOffsetOnAxis(
                        ap=src_sb[:, col : col + 1], axis=0
                    ),
                )
                a_t = ap.tile([128, WIN], f32, tag="A")
                nc.vector.tensor_scalar(
                    a_t[:],
                    iota_sb[:],
                    dst_sb[:, col : col + 1],
                    w_sb[:, col : col + 1],
                    mybir.AluOpType.is_equal,
                    mybir.AluOpType.mult,
                )
                nc.tensor.matmul(
                    out=acc[:],
                    lhsT=a_t[:],
                    rhs=g[:],
                    start=(j == 0),
                    stop=(j == c - 1),
                )
            nf = wk.tile([WIN, D], f32, tag="nf")
            nc.sync.dma_start(out=nf[:], in_=nfown_d[t * WIN : (t + 1) * WIN, :])
            x = wk.tile([WIN, D], f32, tag="x")
            nc.vector.tensor_tensor(
                out=x[:], in0=nf[:], in1=acc[:], op=mybir.AluOpType.mult
            )
            xt_ps = ps.tile([D, WIN], f32, tag="xt")
            nc.tensor.transpose(out=xt_ps[:], in_=x[:], identity=ident_sb[:])
            xt = wk.tile([D, WIN], f32, tag="xts")
            nc.scalar.activation(
                out=xt[:], in_=xt_ps[:], func=mybir.ActivationFunctionType.Copy
            )
            op_ps = ps.tile([WIN, D], f32, tag="op")
            nc.tensor.matmul(
                out=op_ps[:], lhsT=xt[:], rhs=wt_sb[:], start=True, stop=True
            )
            ob = wk.tile([WIN, D], f32, tag="ob")
            nc.scalar.activation(
                out=ob[:],
                in_=op_ps[:],
                func=mybir.ActivationFunctionType.Lrelu,
                alpha=0.01,
            )
            nc.sync.dma_start(out=out_d[t * WIN : (t + 1) * WIN, :], in_=ob[:])
            start += c
    _split_excess_waits(nc)
    return nc


def _kernel_impl(nfeat, edge_src, edge_dst, edge_w, W, npc, trace=False):
    n, d = nfeat.shape
    assert d == D and npc * N_CORES == n
    nw = (npc + WIN - 1) // WIN

    order = np.argsort(edge_dst, kind="stable")
    ds = edge_dst[order].astype(np.int64)
    ss = edge_src[order].astype(np.int64)
    ws = edge_w[order].astype(np.float32)

    bounds = []
    for k in range(N_CORES):
        base = k * npc
        for t in range(nw):
            bounds.append(min(base + t * WIN, base + npc))
    bounds.append(N_CORES * npc)
    idx = np.searchsorted(ds, np.array(bounds))
    cnts = np.diff(idx).reshape(N_CORES, nw)
    pos = idx[:-1].reshape(N_CORES, nw)

    c_list = [int(max(1, v)) for v in np.ceil(cnts / 128).max(axis=0).astype(int)]
    ct = int(sum(c_list))
    starts = np.concatenate([[0], np.cumsum(c_list)[:-1]]).astype(int)

    src_arr = np.zeros((N_CORES, 128, ct), np.int32)
    dst_arr = np.zeros((N_CORES, 128, ct), np.float32)
    w_arr = np.zeros((N_CORES, 128, ct), np.float32)
    for k in range(N_CORES):
        for t in range(nw):
            cnt = int(cnts[k, t])
            if cnt == 0:
                continue
            o0 = int(pos[k, t])
            j = np.arange(cnt)
            col = starts[t] + (j // 128)
            row = j % 128
            src_arr[k, row, col] = ss[o0 : o0 + cnt]
            dst_arr[k, row, col] = (ds[o0 : o0 + cnt] - (k * npc + t * WIN)).astype(
                np.float32
            )
            w_arr[k, row, col] = ws[o0 : o0 + cnt]

    wt = np.ascontiguousarray(W.T.astype(np.float32))
    iota = np.tile(np.arange(WIN, dtype=np.float32), (128, 1))
    ident = np.eye(128, dtype=np.float32)
    nfeat = np.ascontiguousarray(nfeat.astype(np.float32))

    key = (n, npc, ct, tuple(c_list))
    if key not in _nc_cache:
        _nc_cache[key] = _build_nc(n, nw, ct, c_list)
    nc = _nc_cache[key]

    in_maps = []
    for k in range(N_CORES):
        nfown = np.zeros((nw * WIN, D), np.float32)
        lo = k * npc
        avail = min(nw * WIN, n - lo)
        nfown[:avail] = nfeat[lo : lo + avail]
        in_maps.append(
            {
                "nfeat": nfeat,
                "nfown": nfown,
                "src": src_arr[k],
                "dstf": dst_arr[k],
                "wf": w_arr[k],
                "wt": wt,
                "iota": iota,
                "ident": ident,
            }
        )

    r = run_bass_kernel_spmd(nc, in_maps, list(range(N_CORES)), trace=trace)
    out = np.empty((n, D), np.float32)
    for k in range(N_CORES):
        out[k * npc : (k + 1) * npc] = r.results[k]["out"][:npc]
    if trace:
        return out, r
    return out


def kernel(nfeat, edge_src, edge_dst, edge_w, W):
    return _kernel_impl(
        np.asarray(nfeat),
        np.asarray(edge_src),
        np.asarray(edge_dst),
        np.asarray(edge_w),
        np.asarray(W),
        npc=12500,
    )



# revision 6
# speedup vs baseline: 8.1309x; 8.1309x over previous
"""KGATConv GNN message-passing kernel for 8 Trainium2 NeuronCores.

Strategy (dst-node ownership; fp16 staging + on-device AllGather):
  - Core k owns nodes [k*12500, (k+1)*12500).  Host stages only core k's
    own feature rows (fp16, padded to 12544); the full gather table is
    rebuilt on-device with an HBM AllGather over NeuronLink -- host->device
    traffic for nfeat drops 16x vs replicating f32 to all cores.
  - Host sorts edges by dst and buckets per (core, 128-node window), padding
    each window's edge run to whole 128-edge chunks (chunk counts shared
    across cores so all 8 run one SPMD program).  Edge payload ships fp16
    (dst window offsets, weights, W^T) in one array + int32 src in another.
  - Device, per chunk: indirect-DMA gather of 128 fp16 rows (one offset per
    partition); DVE builds A[p,j] = w_p * (dst_p == j) in fp16; PE fp16
    matmul-accumulates h_nb = A^T @ msg in f32 PSUM.  Finalize per window:
    X = nfeat_own * h_nb, X^T via PE transpose, out = X @ W^T on PE (fp16),
    LeakyReLU on ACT, fp16 DMA out.
  - The PJRT executable, donated-zero outputs, and NEFF are all cached at
    module level so warm calls pay only staging + exec + fetch.
"""

import sys

sys.path.insert(0, "/opt/trn_rl_repo")

from contextlib import ExitStack

import numpy as np

import concourse.bass as bass
import concourse.mybir as mybir
import concourse.tile as tile

N_CORES = 8
D = 128
WIN = 128
NPC = 12500
NWIN = (NPC + WIN - 1) // WIN  # 98
PADN = NWIN * WIN  # 12544
GN = N_CORES * PADN  # 100352

_cache = {}


def _split_excess_waits(nc, maxw=1):
    # This walrus build rejects instructions carrying more than one sync
    # wait; move extras onto preceding single-wait NoOps on the same engine.
    for f in nc.m.functions:
        for bb in f.blocks:
            out = []
            for inst in bb.instructions:
                si = inst.sync_info
                waits = list(si.on_wait) if si and si.on_wait else []
                if len(waits) > maxw:
                    extra, keep = waits[:-maxw], waits[-maxw:]
                    for i in range(0, len(extra), maxw):
                        nop = mybir.InstNoOp(
                            name=nc.get_next_instruction_name(), ins=[], outs=[]
                        )
                        nop.engine = inst.engine
                        nop.sync_info = type(si)(
                            on_wait=extra[i : i + maxw], on_update=[]
                        )
                        nc.register_instruction(nop, overwrite=True)
                        out.append(nop)
                    si.on_wait = keep
                out.append(inst)
            bb.instructions[:] = out


def _build_nc(ct, c_list):
    f16 = mybir.dt.float16
    f32 = mybir.dt.float32
    nc = bass.Bass()
    nfsh_d = nc.declare_dram_parameter("nfsh", [PADN, D], f16, isOutput=False)
    edgf_d = nc.declare_dram_parameter("edgf", [256, ct + 128], f16, isOutput=False)
    srci_d = nc.declare_dram_parameter("srci", [128, ct], mybir.dt.int32, isOutput=False)
    out_d = nc.declare_dram_parameter("out", [PADN, D], f16, isOutput=True)
    iota_d = nc.inline_tensor(
        np.tile(np.arange(WIN, dtype=np.float32), (128, 1)), name="iota"
    )
    ident_d = nc.inline_tensor(np.eye(128, dtype=np.float32), name="ident")

    with tile.TileContext(nc) as tc, ExitStack() as ctx:
        dram = ctx.enter_context(tc.tile_pool(name="dram", bufs=1, space="DRAM"))
        agin = dram.tile([PADN, D], f16)
        agout = dram.tile([GN, D], f16, addr_space="Shared")
        const = ctx.enter_context(tc.tile_pool(name="const", bufs=1))
        gp = ctx.enter_context(tc.tile_pool(name="gp", bufs=10))
        ap_pool = ctx.enter_context(tc.tile_pool(name="ap", bufs=4))
        wk = ctx.enter_context(tc.tile_pool(name="wk", bufs=3))
        ps = ctx.enter_context(tc.tile_pool(name="ps", bufs=2, space="PSUM"))

        # kick off the gather-table rebuild first; const loads overlap it
        nc.sync.dma_start(out=agin[:], in_=nfsh_d[:])
        nc.gpsimd.collective_compute(
            "AllGather",
            mybir.AluOpType.bypass,
            replica_groups=[list(range(N_CORES))],
            ins=[agin.opt()],
            outs=[agout.opt()],
        )

        srci_sb = const.tile([128, ct], mybir.dt.int32)
        nc.sync.dma_start(out=srci_sb[:], in_=srci_d[:])
        dst_sb16 = const.tile([128, ct], f16)
        nc.sync.dma_start(out=dst_sb16[:], in_=edgf_d[0:128, 0:ct])
        w_sb16 = const.tile([128, ct], f16)
        nc.sync.dma_start(out=w_sb16[:], in_=edgf_d[128:256, 0:ct])
        # DVE scalar operands must be f32; widen once on device
        dst_sb = const.tile([128, ct], f32)
        nc.scalar.activation(
            out=dst_sb[:], in_=dst_sb16[:], func=mybir.ActivationFunctionType.Copy
        )
        w_sb = const.tile([128, ct], f32)
        nc.scalar.activation(
            out=w_sb[:], in_=w_sb16[:], func=mybir.ActivationFunctionType.Copy
        )
        wt_sb = const.tile([128, 128], f16)
        nc.sync.dma_start(out=wt_sb[:], in_=edgf_d[0:128, ct : ct + 128])
        iota_sb = const.tile([128, WIN], f32)
        nc.sync.dma_start(out=iota_sb[:], in_=iota_d[:])
        ident_sb = const.tile([128, 128], f32)
        nc.sync.dma_start(out=ident_sb[:], in_=ident_d[:])

        start = 0
        for t in range(NWIN):
            c = c_list[t]
            acc = ps.tile([WIN, D], f32, tag="acc")
            for j in range(c):
                col = start + j
                # one offset per partition; dest [128,128] = one table row
                # per partition (the only indirect mode this walrus build
                # executes correctly).
                g = gp.tile([128, D], f16, tag="g")
                nc.gpsimd.indirect_dma_start(
                    out=g[:],
                    out_offset=None,
                    in_=agout[:],
                    in_offset=bass.IndirectOffsetOnAxis(
                        ap=srci_sb[:, col : col + 1], axis=0
                    ),
                )
                a_t = ap_pool.tile([128, WIN], f16, tag="A")
                nc.vector.tensor_scalar(
                    a_t[:],
                    iota_sb[:],
                    dst_sb[:, col : col + 1],
                    w_sb[:, col : col + 1],
                    mybir.AluOpType.is_equal,
                    mybir.AluOpType.mult,
                )
                nc.tensor.matmul(
                    out=acc[:],
                    lhsT=a_t[:],
                    rhs=g[:],
                    start=(j == 0),
                    stop=(j == c - 1),
                )
            nf = wk.tile([WIN, D], f16, tag="nf")
            nc.sync.dma_start(out=nf[:], in_=nfsh_d[t * WIN : (t + 1) * WIN, :])
            x = wk.tile([WIN, D], f32, tag="x")
            nc.vector.tensor_tensor(
                out=x[:], in0=nf[:], in1=acc[:], op=mybir.AluOpType.mult
            )
            xt_ps = ps.tile([D, WIN], f32, tag="xt")
            nc.tensor.transpose(out=xt_ps[:], in_=x[:], identity=ident_sb[:])
            xt = wk.tile([D, WIN], f16, tag="xts")
            nc.scalar.activation(
                out=xt[:], in_=xt_ps[:], func=mybir.ActivationFunctionType.Copy
            )
            op_ps = ps.tile([WIN, D], f32, tag="op")
            nc.tensor.matmul(
                out=op_ps[:], lhsT=xt[:], rhs=wt_sb[:], start=True, stop=True
            )
            ob = wk.tile([WIN, D], f16, tag="ob")
            nc.scalar.activation(
                out=ob[:],
                in_=op_ps[:],
                func=mybir.ActivationFunctionType.Lrelu,
                alpha=0.01,
            )
            nc.sync.dma_start(out=out_d[t * WIN : (t + 1) * WIN, :], in_=ob[:])
            start += c
    _split_excess_waits(nc)
    return nc


def _get_exec(ct, c_list):
    key = (ct, tuple(c_list))
    if key in _cache:
        return _cache[key]

    import jax
    import jax.numpy as jnp
    from jax.sharding import Mesh, NamedSharding, PartitionSpec
    from jax.experimental.shard_map import shard_map
    from concourse.bass2jax import (
        _bass_exec_p,
        install_neuronx_cc_hook,
        partition_id_tensor,
    )

    install_neuronx_cc_hook()
    nc = _build_nc(ct, c_list)
    assert nc.dbg_addr is None
    partition_name = nc.partition_id_tensor.name if nc.partition_id_tensor else None

    in_names = []
    out_names = []
    out_avals = []
    for alloc in nc.m.functions[0].allocations:
        if not isinstance(alloc, mybir.MemoryLocationSet):
            continue
        name = alloc.memorylocations[0].name
        if alloc.kind == "ExternalInput":
            if name != partition_name:
                in_names.append(name)
        elif alloc.kind == "ExternalOutput":
            out_names.append(name)
            out_avals.append(
                jax.core.ShapedArray(
                    tuple(alloc.tensor_shape), mybir.dt.np(alloc.dtype)
                )
            )
    assert in_names == ["nfsh", "edgf", "srci"], in_names
    assert out_names == ["out"], out_names
    n_params = len(in_names)
    all_names = in_names + out_names
    if partition_name is not None:
        all_names.append(partition_name)
    all_names = tuple(all_names)

    def _body(*args):
        operands = list(args)
        if partition_name is not None:
            operands.append(partition_id_tensor())
        return tuple(
            _bass_exec_p.bind(
                *operands,
                out_avals=tuple(out_avals),
                in_names=all_names,
                out_names=tuple(out_names),
                lowering_input_output_aliases=(),
                sim_require_finite=True,
                sim_require_nnan=True,
                nc=nc,
            )
        )

    devices = jax.devices()[:N_CORES]
    mesh = Mesh(np.asarray(devices), ("core",))
    sh = NamedSharding(mesh, PartitionSpec("core"))
    sharded = jax.jit(
        shard_map(
            _body,
            mesh=mesh,
            in_specs=(PartitionSpec("core"),) * (n_params + 1),
            out_specs=(PartitionSpec("core"),),
            check_rep=False,
        ),
        donate_argnums=(n_params,),
        keep_unused=True,
    )
    zeros_fn = jax.jit(
        lambda: jnp.zeros((N_CORES * PADN, D), jnp.float16), out_shardings=sh
    )
    _cache[key] = (sharded, zeros_fn, sh)
    return _cache[key]


def _kernel_impl(nfeat, edge_src, edge_dst, edge_w, W):
    import jax

    n, d = nfeat.shape
    assert d == D and n == N_CORES * NPC

    # Stage the feature shards first so the transfer overlaps edge binning.
    nfg = np.zeros((GN, D), np.float16)
    nf16 = nfeat.astype(np.float16)
    for k in range(N_CORES):
        nfg[k * PADN : k * PADN + NPC] = nf16[k * NPC : (k + 1) * NPC]
    devices = None  # device_put deferred until sharding known (cache warm path)
    nfg_dev = None
    if _cache:
        sh = next(iter(_cache.values()))[2]
        nfg_dev = jax.device_put(nfg, sh)

    order = np.argsort(edge_dst, kind="stable")
    ds = edge_dst[order].astype(np.int64)
    ss = edge_src[order].astype(np.int64)
    ws16 = edge_w[order].astype(np.float16)
    ss_remap = ((ss // NPC) * PADN + (ss % NPC)).astype(np.int32)

    bounds = []
    for k in range(N_CORES):
        base = k * NPC
        for t in range(NWIN):
            bounds.append(min(base + t * WIN, base + NPC))
    bounds.append(N_CORES * NPC)
    idx = np.searchsorted(ds, np.array(bounds))
    cnts = np.diff(idx).reshape(N_CORES, NWIN)
    pos = idx[:-1].reshape(N_CORES, NWIN)

    c_list = [int(max(1, v)) for v in np.ceil(cnts / 128).max(axis=0).astype(int)]
    ct = int(sum(c_list))
    starts = np.concatenate([[0], np.cumsum(c_list)[:-1]]).astype(int)

    srci_g = np.zeros((N_CORES * 128, ct), np.int32)
    edgf_g = np.zeros((N_CORES * 256, ct + 128), np.float16)
    wt16 = np.ascontiguousarray(W.T.astype(np.float16))
    for k in range(N_CORES):
        for t in range(NWIN):
            cnt = int(cnts[k, t])
            if cnt == 0:
                continue
            o0 = int(pos[k, t])
            j = np.arange(cnt)
            col = starts[t] + (j // 128)
            row = j % 128
            srci_g[k * 128 + row, col] = ss_remap[o0 : o0 + cnt]
            edgf_g[k * 256 + row, col] = (
                ds[o0 : o0 + cnt] - (k * NPC + t * WIN)
            ).astype(np.float16)
            edgf_g[k * 256 + 128 + row, col] = ws16[o0 : o0 + cnt]
        edgf_g[k * 256 : k * 256 + 128, ct : ct + 128] = wt16

    sharded, zeros_fn, sh = _get_exec(ct, c_list)
    if nfg_dev is None:
        nfg_dev = jax.device_put(nfg, sh)
    edgf_dev = jax.device_put(edgf_g, sh)
    srci_dev = jax.device_put(srci_g, sh)
    (out_arr,) = sharded(nfg_dev, edgf_dev, srci_dev, zeros_fn())
    out16 = np.asarray(out_arr).reshape(N_CORES, PADN, D)[:, :NPC]
    return out16.reshape(n, D).astype(np.float32)


def kernel(nfeat, edge_src, edge_dst, edge_w, W):
    return _kernel_impl(
        np.asarray(nfeat),
        np.asarray(edge_src),
        np.asarray(edge_dst),
        np.asarray(edge_w),
        np.asarray(W),
    )


# revision 9
# speedup vs baseline: 8.7053x; 1.0706x over previous
"""KGATConv GNN message-passing kernel for 8 Trainium2 NeuronCores.

Strategy (dst-node ownership; fp16 staging + on-device AllGather):
  - Core k owns nodes [k*12500, (k+1)*12500).  Host stages only core k's
    own feature rows (fp16, padded to 12544); the full gather table is
    rebuilt on-device with an HBM AllGather over NeuronLink -- host->device
    traffic for nfeat drops 16x vs replicating f32 to all cores.
  - Host sorts edges by dst and buckets per (core, 128-node window), padding
    each window's edge run to whole 128-edge chunks (chunk counts shared
    across cores so all 8 run one SPMD program).  Edge payload ships fp16
    (dst window offsets, weights, W^T) in one array + int32 src in another.
  - Device, per chunk: indirect-DMA gather of 128 fp16 rows (one offset per
    partition); DVE builds A[p,j] = w_p * (dst_p == j) in fp16; PE fp16
    matmul-accumulates h_nb = A^T @ msg in f32 PSUM.  Finalize per window:
    X = nfeat_own * h_nb, X^T via PE transpose, out = X @ W^T on PE (fp16),
    LeakyReLU on ACT, fp16 DMA out.
  - The PJRT executable, donated-zero outputs, and NEFF are all cached at
    module level so warm calls pay only staging + exec + fetch.
"""

import sys

sys.path.insert(0, "/opt/trn_rl_repo")

from contextlib import ExitStack

import numpy as np

import concourse.bass as bass
import concourse.mybir as mybir
import concourse.tile as tile

N_CORES = 8
D = 128
WIN = 128
NPC = 12500
NWIN = (NPC + WIN - 1) // WIN  # 98
PADN = NWIN * WIN  # 12544
GN = N_CORES * PADN  # 100352

_cache = {}


def _split_excess_waits(nc, maxw=1):
    # This walrus build rejects instructions carrying more than one sync
    # wait; move extras onto preceding single-wait NoOps on the same engine.
    for f in nc.m.functions:
        for bb in f.blocks:
            out = []
            for inst in bb.instructions:
                si = inst.sync_info
                waits = list(si.on_wait) if si and si.on_wait else []
                if len(waits) > maxw:
                    extra, keep = waits[:-maxw], waits[-maxw:]
                    for i in range(0, len(extra), maxw):
                        nop = mybir.InstNoOp(
                            name=nc.get_next_instruction_name(), ins=[], outs=[]
                        )
                        nop.engine = inst.engine
                        nop.sync_info = type(si)(
                            on_wait=extra[i : i + maxw], on_update=[]
                        )
                        nc.register_instruction(nop, overwrite=True)
                        out.append(nop)
                    si.on_wait = keep
                out.append(inst)
            bb.instructions[:] = out


def _build_nc(ct, c_list):
    f16 = mybir.dt.float16
    f32 = mybir.dt.float32
    nc = bass.Bass()
    nfsh_d = nc.declare_dram_parameter("nfsh", [PADN, D], f16, isOutput=False)
    edgf_d = nc.declare_dram_parameter("edgf", [256, ct + 128], f16, isOutput=False)
    srci_d = nc.declare_dram_parameter("srci", [128, ct], mybir.dt.int32, isOutput=False)
    out_d = nc.declare_dram_parameter("out", [PADN, D], f16, isOutput=True)
    iota_d = nc.inline_tensor(
        np.tile(np.arange(WIN, dtype=np.float32), (128, 1)), name="iota"
    )
    ident_d = nc.inline_tensor(np.eye(128, dtype=np.float32), name="ident")

    with tile.TileContext(nc) as tc, ExitStack() as ctx:
        dram = ctx.enter_context(tc.tile_pool(name="dram", bufs=1, space="DRAM"))
        agin = dram.tile([PADN, D], f16)
        agout = dram.tile([GN, D], f16, addr_space="Shared")
        const = ctx.enter_context(tc.tile_pool(name="const", bufs=1))
        gp = ctx.enter_context(tc.tile_pool(name="gp", bufs=10))
        ap_pool = ctx.enter_context(tc.tile_pool(name="ap", bufs=4))
        wk = ctx.enter_context(tc.tile_pool(name="wk", bufs=3))
        ps = ctx.enter_context(tc.tile_pool(name="ps", bufs=2, space="PSUM"))

        # kick off the gather-table rebuild first; const loads overlap it
        nc.sync.dma_start(out=agin[:], in_=nfsh_d[:])
        nc.gpsimd.collective_compute(
            "AllGather",
            mybir.AluOpType.bypass,
            replica_groups=[list(range(N_CORES))],
            ins=[agin.opt()],
            outs=[agout.opt()],
        )

        srci_sb = const.tile([128, ct], mybir.dt.int32)
        nc.sync.dma_start(out=srci_sb[:], in_=srci_d[:])
        dst_sb16 = const.tile([128, ct], f16)
        nc.sync.dma_start(out=dst_sb16[:], in_=edgf_d[0:128, 0:ct])
        w_sb16 = const.tile([128, ct], f16)
        nc.sync.dma_start(out=w_sb16[:], in_=edgf_d[128:256, 0:ct])
        # DVE scalar operands must be f32; widen once on device
        dst_sb = const.tile([128, ct], f32)
        nc.scalar.activation(
            out=dst_sb[:], in_=dst_sb16[:], func=mybir.ActivationFunctionType.Copy
        )
        w_sb = const.tile([128, ct], f32)
        nc.scalar.activation(
            out=w_sb[:], in_=w_sb16[:], func=mybir.ActivationFunctionType.Copy
        )
        wt_sb = const.tile([128, 128], f16)
        nc.sync.dma_start(out=wt_sb[:], in_=edgf_d[0:128, ct : ct + 128])
        iota_sb = const.tile([128, WIN], f32)
        nc.sync.dma_start(out=iota_sb[:], in_=iota_d[:])
        ident_sb = const.tile([128, 128], f32)
        nc.sync.dma_start(out=ident_sb[:], in_=ident_d[:])

        start = 0
        for t in range(NWIN):
            c = c_list[t]
            acc = ps.tile([WIN, D], f32, tag="acc")
            for j in range(c):
                col = start + j
                # one offset per partition; dest [128,128] = one table row
                # per partition (the only indirect mode this walrus build
                # executes correctly).
                g = gp.tile([128, D], f16, tag="g")
                nc.gpsimd.indirect_dma_start(
                    out=g[:],
                    out_offset=None,
                    in_=agout[:],
                    in_offset=bass.IndirectOffsetOnAxis(
                        ap=srci_sb[:, col : col + 1], axis=0
                    ),
                )
                a_t = ap_pool.tile([128, WIN], f16, tag="A")
                nc.vector.tensor_scalar(
                    a_t[:],
                    iota_sb[:],
                    dst_sb[:, col : col + 1],
                    w_sb[:, col : col + 1],
                    mybir.AluOpType.is_equal,
                    mybir.AluOpType.mult,
                )
                nc.tensor.matmul(
                    out=acc[:],
                    lhsT=a_t[:],
                    rhs=g[:],
                    start=(j == 0),
                    stop=(j == c - 1),
                )
            nf = wk.tile([WIN, D], f16, tag="nf")
            nc.sync.dma_start(out=nf[:], in_=nfsh_d[t * WIN : (t + 1) * WIN, :])
            x = wk.tile([WIN, D], f32, tag="x")
            nc.vector.tensor_tensor(
                out=x[:], in0=nf[:], in1=acc[:], op=mybir.AluOpType.mult
            )
            xt_ps = ps.tile([D, WIN], f32, tag="xt")
            nc.tensor.transpose(out=xt_ps[:], in_=x[:], identity=ident_sb[:])
            xt = wk.tile([D, WIN], f16, tag="xts")
            nc.scalar.activation(
                out=xt[:], in_=xt_ps[:], func=mybir.ActivationFunctionType.Copy
            )
            op_ps = ps.tile([WIN, D], f32, tag="op")
            nc.tensor.matmul(
                out=op_ps[:], lhsT=xt[:], rhs=wt_sb[:], start=True, stop=True
            )
            ob = wk.tile([WIN, D], f16, tag="ob")
            nc.scalar.activation(
                out=ob[:],
                in_=op_ps[:],
                func=mybir.ActivationFunctionType.Lrelu,
                alpha=0.01,
            )
            nc.sync.dma_start(out=out_d[t * WIN : (t + 1) * WIN, :], in_=ob[:])
            start += c
    _split_excess_waits(nc)
    return nc


def _get_exec(ct, c_list):
    key = (ct, tuple(c_list))
    if key in _cache:
        return _cache[key]

    import jax
    import jax.numpy as jnp
    from jax.sharding import Mesh, NamedSharding, PartitionSpec
    from jax.experimental.shard_map import shard_map
    from concourse.bass2jax import (
        _bass_exec_p,
        install_neuronx_cc_hook,
        partition_id_tensor,
    )

    install_neuronx_cc_hook()
    nc = _build_nc(ct, c_list)
    assert nc.dbg_addr is None
    partition_name = nc.partition_id_tensor.name if nc.partition_id_tensor else None

    in_names = []
    out_names = []
    out_avals = []
    for alloc in nc.m.functions[0].allocations:
        if not isinstance(alloc, mybir.MemoryLocationSet):
            continue
        name = alloc.memorylocations[0].name
        if alloc.kind == "ExternalInput":
            if name != partition_name:
                in_names.append(name)
        elif alloc.kind == "ExternalOutput":
            out_names.append(name)
            out_avals.append(
                jax.core.ShapedArray(
                    tuple(alloc.tensor_shape), mybir.dt.np(alloc.dtype)
                )
            )
    assert in_names == ["nfsh", "edgf", "srci"], in_names
    assert out_names == ["out"], out_names
    n_params = len(in_names)
    all_names = in_names + out_names
    if partition_name is not None:
        all_names.append(partition_name)
    all_names = tuple(all_names)

    def _body(*args):
        operands = list(args)
        if partition_name is not None:
            operands.append(partition_id_tensor())
        return tuple(
            _bass_exec_p.bind(
                *operands,
                out_avals=tuple(out_avals),
                in_names=all_names,
                out_names=tuple(out_names),
                lowering_input_output_aliases=(),
                sim_require_finite=True,
                sim_require_nnan=True,
                nc=nc,
            )
        )

    devices = jax.devices()[:N_CORES]
    mesh = Mesh(np.asarray(devices), ("core",))
    sh = NamedSharding(mesh, PartitionSpec("core"))
    sharded = jax.jit(
        shard_map(
            _body,
            mesh=mesh,
            in_specs=(PartitionSpec("core"),) * (n_params + 1),
            out_specs=(PartitionSpec("core"),),
            check_rep=False,
        ),
        donate_argnums=(n_params,),
        keep_unused=True,
    )
    zeros_fn = jax.jit(
        lambda: jnp.zeros((N_CORES * PADN, D), jnp.float16), out_shardings=sh
    )
    _cache[key] = (sharded, zeros_fn, sh)
    return _cache[key]


def _kernel_impl(nfeat, edge_src, edge_dst, edge_w, W):
    import jax

    n, d = nfeat.shape
    assert d == D and n == N_CORES * NPC

    # Stage the feature shards first so the transfer overlaps edge binning.
    nfg = np.zeros((GN, D), np.float16)
    nfg.reshape(N_CORES, PADN, D)[:, :NPC] = nfeat.reshape(N_CORES, NPC, D)
    nfg_dev = None
    if _cache:
        sh = next(iter(_cache.values()))[2]
        nfg_dev = jax.device_put(nfg, sh)

    # Bucket edges by (dst core, 128-node window).  Sorting a uint16 window
    # key is ~2x faster than sorting the raw int32 dst.
    kd = edge_dst // NPC
    r = edge_dst - kd * NPC
    tw = r // WIN
    off16 = (r - tw * WIN).astype(np.float16)
    key = (kd * NWIN + tw).astype(np.uint16)
    order = np.argsort(key, kind="stable")
    key_s = key[order]
    ks = edge_src // NPC
    srcr = (ks * PADN + (edge_src - ks * NPC)).astype(np.int32)
    srcr_s = srcr[order]
    off_s = off16[order]
    w_s = edge_w[order].astype(np.float16)

    idx = np.searchsorted(key_s, np.arange(N_CORES * NWIN + 1))
    cnts = np.diff(idx).reshape(N_CORES, NWIN)
    pos = idx[:-1].reshape(N_CORES, NWIN)

    c_list = [int(max(1, v)) for v in np.ceil(cnts / 128).max(axis=0).astype(int)]
    ct = int(sum(c_list))
    starts = np.concatenate([[0], np.cumsum(c_list)[:-1]]).astype(int)

    srci_g = np.zeros((N_CORES * 128, ct), np.int32)
    edgf_g = np.zeros((N_CORES * 256, ct + 128), np.float16)
    wt16 = np.ascontiguousarray(W.T.astype(np.float16))
    for k in range(N_CORES):
        for t in range(NWIN):
            cnt = int(cnts[k, t])
            if cnt == 0:
                continue
            o0 = int(pos[k, t])
            j = np.arange(cnt)
            col = starts[t] + (j // 128)
            row = j % 128
            srci_g[k * 128 + row, col] = srcr_s[o0 : o0 + cnt]
            edgf_g[k * 256 + row, col] = off_s[o0 : o0 + cnt]
            edgf_g[k * 256 + 128 + row, col] = w_s[o0 : o0 + cnt]
        edgf_g[k * 256 : k * 256 + 128, ct : ct + 128] = wt16

    sharded, zeros_fn, sh = _get_exec(ct, c_list)
    if nfg_dev is None:
        nfg_dev = jax.device_put(nfg, sh)
    edgf_dev = jax.device_put(edgf_g, sh)
    srci_dev = jax.device_put(srci_g, sh)
    (out_arr,) = sharded(nfg_dev, edgf_dev, srci_dev, zeros_fn())
    out16 = np.asarray(out_arr).reshape(N_CORES, PADN, D)[:, :NPC]
    return out16.reshape(n, D).astype(np.float32)


def kernel(nfeat, edge_src, edge_dst, edge_w, W):
    return _kernel_impl(
        np.asarray(nfeat),
        np.asarray(edge_src),
        np.asarray(edge_dst),
        np.asarray(edge_w),
        np.asarray(W),
    )
